# revision 1
# baseline (speedup 1.0000x reference)
"""Trainium2 Bass kernel for nn_CopyGenerator (scatter_memory).

Computation (see the reference):
  out_tgt = log_softmax(hidden @ W.T + b)                    [T,B,VT]
  gate1m  = 1 - sigmoid(dec @ Wc.T + bc)                     [T,B]
  ext[t,b,v] = gate1m[t,b] * sum_s attn[t,b,s]*(idx[s,b]==v), idx==UNK masked
  out_ext = log(clip(ext, 1e-3, 1-1e-3))                     [T,B,VE]
  out = concat([out_tgt, out_ext], -1)

Sharding (8 cores):
  - Big matmul + log_softmax: column-parallel over tgt vocab (each core owns a
    4000-wide W slice, SBUF-resident bf16; all 3200 rows). The softmax
    denominator needs the full-vocab sum -> per-chunk partial row sums are
    AllReduced across cores (5 tiny pipelined collectives).
  - Copy-gate + scatter-add over ext vocab: data-parallel over batch (8 batch
    elements per core). The scatter-add is aw.T @ onehot(idx) on the tensor
    engine (exact, handles duplicate indices); attn is fed as a bf16 hi/lo
    pair so the ext sums are fp32-accurate.
"""

import sys

if "/opt/trn_rl_repo" not in sys.path:
    sys.path.insert(0, "/opt/trn_rl_repo")

from contextlib import ExitStack

import ml_dtypes
import numpy as np

import concourse.bass as bass
import concourse.mybir as mybir
import concourse.tile as tile
from concourse import bacc
from concourse.bass_utils import run_bass_kernel_spmd

F32 = mybir.dt.float32
BF16 = mybir.dt.bfloat16
AF = mybir.ActivationFunctionType
OP = mybir.AluOpType

T, B, S, H = 50, 64, 100, 512
VT, VE = 32000, 5000
N_CORES = 8
VL = VT // N_CORES       # 4000 vocab cols per core
BL = B // N_CORES        # 8 batch per core (ext part)
R = T * B                # 3200 rows
RL = T * BL              # 400 rows (ext part)
KT = H // 128            # 4 k-tiles
MT = R // 128            # 25 m-tiles
CH = 5                   # m-tiles per lse chunk
NCH = MT // CH           # 5 chunks
NW = 500                 # main n-block width (<=512 f32 per psum bank)
NQ = 4                   # psum pairs per m-tile (2 n-blocks each)
EXT_N = 500
EXT_NB = VE // EXT_N     # 10

LOG_LO = float(np.log(0.001))
LOG_HI = float(np.log(1.0 - 0.001))

_CACHE = {}


def _dedupe_act_table_loads(nc):
    """Collapse activation-table thrash: point every load at a table that
    serves its following activations when one exists, then drop loads that
    re-load the already-loaded table. Saves ~1.8us per removed load on ACT."""
    from concourse.hw_specs import get_activation_tables
    tables = list(get_activation_tables(nc.m.arch).items())
    for blk in nc.m.functions[0].blocks:
        insts = blk.instructions
        loads = [(i, inst) for i, inst in enumerate(insts)
                 if isinstance(inst, mybir.InstLoadActFuncSet)]
        if not loads:
            continue
        for li, (pos, inst) in enumerate(loads):
            end = loads[li + 1][0] if li + 1 < len(loads) else len(insts)
            funcs = {s.func for s in insts[pos:end]
                     if isinstance(s, mybir.InstActivation)}
            if not funcs:
                continue
            want = funcs | {AF.Exp, AF.Ln, AF.Identity, AF.Copy}
            pick = None
            for tid, (name, fs) in enumerate(tables):
                if want <= fs:
                    pick = tid
                    break
            if pick is None:
                for tid, (name, fs) in enumerate(tables):
                    if funcs <= fs:
                        pick = tid
                        break
            if pick is not None:
                inst.act_func_set_id = pick
        cur = None
        to_drop = []
        for pos, inst in loads:
            if cur is not None and inst.act_func_set_id == cur:
                si = inst.sync_info
                clean = si is None or (not si.on_wait and not si.on_update)
                if clean:
                    to_drop.append(inst)
                    continue
            cur = inst.act_func_set_id
        for inst in to_drop:
            insts.remove(inst)


def _build(with_bias):
    nc = bacc.Bacc("TRN2", target_bir_lowering=False, debug=False,
                   num_devices=N_CORES)

    hT = [nc.dram_tensor(f"hT{k}", [128, R], BF16, kind="ExternalInput").ap()
          for k in range(KT)]
    wT = [nc.dram_tensor(f"wT{k}", [128, VL], BF16, kind="ExternalInput").ap()
          for k in range(KT)]
    if with_bias:
        brow = nc.dram_tensor("brow", [1, VL], BF16, kind="ExternalInput").ap()
    dT = nc.dram_tensor("dT", [KT, 128, RL], F32, kind="ExternalInput").ap()
    wcT = nc.dram_tensor("wcT", [KT, 128, 1], F32, kind="ExternalInput").ap()
    bc_t = nc.dram_tensor("bc", [1, 1], F32, kind="ExternalInput").ap()
    # attn hi/lo bf16 split: [2, S, BL*T]
    attnT = nc.dram_tensor("attnT", [2, S, BL * T], BF16, kind="ExternalInput").ap()
    idx_t = nc.dram_tensor("idx", [S, BL], F32, kind="ExternalInput").ap()

    out_tgt = nc.dram_tensor("out_tgt", [R, VL], F32, kind="ExternalOutput").ap()
    out_ext = nc.dram_tensor("out_ext", [BL, T, VE], F32, kind="ExternalOutput").ap()

    cc_in = [nc.dram_tensor(f"cc_in{g}", [128, CH], F32).ap() for g in range(NCH)]
    cc_out = [nc.dram_tensor(f"cc_out{g}", [128, CH], F32, addr_space="Shared").ap()
              for g in range(NCH)]

    core_ids = list(range(N_CORES))

    with tile.TileContext(nc) as tc, ExitStack() as ctx:
        const = ctx.enter_context(tc.tile_pool(name="const", bufs=1))
        xpool = ctx.enter_context(tc.tile_pool(name="x", bufs=8))
        epool = ctx.enter_context(tc.tile_pool(name="E", bufs=1))
        outpool = ctx.enter_context(tc.tile_pool(name="out", bufs=3))
        statpool = ctx.enter_context(tc.tile_pool(name="stat", bufs=2))
        ohpool = ctx.enter_context(tc.tile_pool(name="oh", bufs=2))
        extstage = ctx.enter_context(tc.tile_pool(name="exts", bufs=4))
        ps_main = ctx.enter_context(tc.tile_pool(name="psm", bufs=3, space="PSUM"))
        ps_ext = ctx.enter_context(tc.tile_pool(name="pse", bufs=2, space="PSUM"))

        # ---- persistent SBUF loads (per-k tiles so PE can start early) ----
        hT_sb = [const.tile([128, R], BF16, name=f"hts{k}") for k in range(KT)]
        wT_sb = [const.tile([128, VL], BF16, name=f"wts{k}") for k in range(KT)]
        for k in range(KT):
            nc.sync.dma_start(wT_sb[k][:], wT[k])
            nc.sync.dma_start(hT_sb[k][:], hT[k])
        if with_bias:
            b_sb = const.tile([1, VL], BF16)
            nc.sync.dma_start(b_sb[:], brow[:])
            ones_sb = const.tile([1, 128], BF16)
            nc.vector.memset(ones_sb[:], 1.0)
        dT_sb = const.tile([128, KT * RL], F32)
        for k in range(KT):
            nc.sync.dma_start(dT_sb[:, k * RL:(k + 1) * RL], dT[k])
        wcT_sb = const.tile([128, KT], F32)
        for k in range(KT):
            nc.sync.dma_start(wcT_sb[:, k:k + 1], wcT[k])
        bc_sb = const.tile([1, 1], F32)
        nc.sync.dma_start(bc_sb[:], bc_t[:])
        ones50 = const.tile([1, 64], F32)
        nc.vector.memset(ones50[:], 1.0)
        attnT_sb = const.tile([S, 2 * BL * T], BF16)
        nc.sync.dma_start(attnT_sb[:, :BL * T], attnT[0])
        nc.sync.dma_start(attnT_sb[:, BL * T:], attnT[1])
        idx_sb = const.tile([S, BL], F32)
        nc.sync.dma_start(idx_sb[:], idx_t[:])
        iota_sb = const.tile([S, VE], F32)
        nc.gpsimd.iota(iota_sb[:], pattern=[[1, VE]], base=0, channel_multiplier=0,
                       allow_small_or_imprecise_dtypes=True)
        # kill ext-vocab column 0 (UNK): make it unmatchable
        nc.gpsimd.memset(iota_sb[:, 0:1], -1.0)

        # ---- copy gate: g1m[t, b] = 1 - sigmoid(dec[t,b] . Wc + bc) ----
        g1m = const.tile([64, BL], F32)
        for b in range(BL):
            gp = ps_ext.tile([64, EXT_N], F32, tag="eps")
            for k in range(KT):
                lhs = dT_sb[:, k * RL + b: k * RL + b + (T - 1) * BL + 1: BL]
                nc.tensor.matmul(gp[:T, 0:1], lhsT=lhs, rhs=wcT_sb[:, k:k + 1],
                                 start=(k == 0), stop=False)
            nc.tensor.matmul(gp[:T, 0:1], lhsT=ones50[:, :T], rhs=bc_sb[:],
                             start=False, stop=True)
            sig = extstage.tile([64, EXT_N], F32, tag="exts")
            nc.scalar.activation(sig[:T, 0:1], gp[:T, 0:1], AF.Sigmoid)
            nc.vector.tensor_scalar(g1m[:T, b:b + 1], sig[:T, 0:1], -1.0, 1.0,
                                    OP.mult, OP.add)

        # ---- ext part emitter (per local batch element) ----
        def emit_ext(b):
            oh = ohpool.tile([S, VE], BF16)
            nc.gpsimd.tensor_scalar(oh[:], iota_sb[:], idx_sb[:, b:b + 1], None,
                                    OP.is_equal)
            for nb in range(EXT_NB):
                ps = ps_ext.tile([64, EXT_N], F32, tag="eps")
                rhs = oh[:, nb * EXT_N:(nb + 1) * EXT_N]
                nc.tensor.matmul(ps[:T, :], lhsT=attnT_sb[:, b * T:(b + 1) * T],
                                 rhs=rhs, start=True, stop=False)
                nc.tensor.matmul(
                    ps[:T, :],
                    lhsT=attnT_sb[:, BL * T + b * T: BL * T + (b + 1) * T],
                    rhs=rhs, start=False, stop=True)
                st = extstage.tile([64, EXT_N], F32, tag="exts")
                # ext = raw * g1m; gate folded into Ln's per-partition scale,
                # clip done in log space (Ln(0) = -inf clips to LOG_LO)
                nc.scalar.activation(st[:T, :], ps[:T, :], AF.Ln,
                                     scale=g1m[:T, b:b + 1])
                nc.vector.tensor_scalar(st[:T, :], st[:T, :], LOG_LO, LOG_HI,
                                        OP.max, OP.min)
                nc.sync.dma_start(out_ext[b, :, nb * EXT_N:(nb + 1) * EXT_N],
                                  st[:T, :])

        # interleave ext batches between main chunks
        ext_sched = {0: [0, 1], 1: [2, 3], 2: [4, 5], 3: [6], 4: [7]}

        # ---- main: logits, online logsumexp, output ----
        for g in range(NCH):
            sums_g = statpool.tile([128, CH], F32, tag="sums")
            x_tiles = []
            for j in range(CH):
                m = g * CH + j
                x_m = xpool.tile([128, VL], BF16, tag="x")
                x_tiles.append(x_m)
                for q in range(NQ):
                    ps = ps_main.tile([128, 1024], F32)
                    for k in range(KT):
                        for nn in range(2):
                            n = 2 * q + nn
                            last = (k == KT - 1) and not with_bias
                            nc.tensor.matmul(
                                ps[:, nn * 512: nn * 512 + NW],
                                lhsT=hT_sb[k][:, m * 128:(m + 1) * 128],
                                rhs=wT_sb[k][:, n * NW:(n + 1) * NW],
                                start=(k == 0), stop=last)
                    if with_bias:
                        for nn in range(2):
                            n = 2 * q + nn
                            nc.tensor.matmul(
                                ps[:, nn * 512: nn * 512 + NW],
                                lhsT=ones_sb[:],
                                rhs=b_sb[:, n * NW:(n + 1) * NW],
                                start=False, stop=True)
                    # psum pair -> x (bf16), one strided copy per pair
                    src = ps[:].rearrange("p (b n) -> p b n", b=2)[:, :, :NW]
                    dst = x_m[:, q * 2 * NW:(q + 1) * 2 * NW].rearrange(
                        "p (b n) -> p b n", b=2)
                    if q % 2 == 0:
                        nc.vector.tensor_copy(dst, src)
                    else:
                        nc.scalar.copy(dst, src)
                E = epool.tile([128, VL], BF16, tag="E")
                nc.scalar.activation(E[:], x_m[:], AF.Exp,
                                     accum_out=sums_g[:, j:j + 1])

            nc.sync.dma_start(cc_in[g][:], sums_g[:])
            nc.gpsimd.collective_compute(
                "AllReduce", OP.add,
                replica_groups=[core_ids],
                ins=[cc_in[g][:]], outs=[cc_out[g][:]])
            tot_g = statpool.tile([128, CH], F32, tag="tot")
            nc.sync.dma_start(tot_g[:], cc_out[g][:])
            neglse = statpool.tile([128, CH], F32, tag="lse")
            nc.scalar.activation(neglse[:], tot_g[:], AF.Ln)
            nc.vector.tensor_scalar(neglse[:], neglse[:], -1.0, None, OP.mult)

            for j in range(CH):
                m = g * CH + j
                x_m = x_tiles[j]
                for half in range(2):
                    o = outpool.tile([128, VL // 2], F32)
                    src = x_m[:, half * (VL // 2):(half + 1) * (VL // 2)]
                    if half == 0:
                        nc.scalar.activation(o[:], src, AF.Identity,
                                             bias=neglse[:, j:j + 1])
                    else:
                        nc.vector.tensor_scalar(o[:], src, neglse[:, j:j + 1],
                                                None, OP.add)
                    nc.sync.dma_start(
                        out_tgt[m * 128:(m + 1) * 128,
                                half * (VL // 2):(half + 1) * (VL // 2)],
                        o[:])

            for b in ext_sched.get(g, []):
                emit_ext(b)

    nc.compile()
    _dedupe_act_table_loads(nc)
    return nc


def _get_nc(with_bias=False):
    key = ("nc", with_bias)
    if key not in _CACHE:
        _CACHE[key] = _build(with_bias)
    return _CACHE[key]


def kernel(**inputs):
    hidden = np.asarray(inputs["hidden"], dtype=np.float32)
    dec = np.asarray(inputs["dec_rnn_output"], dtype=np.float32)
    attn = np.asarray(inputs["attn"], dtype=np.float32)
    c2e = np.asarray(inputs["copy_to_ext"])
    W = np.asarray(inputs["W"], dtype=np.float32)
    bvec = np.asarray(inputs["b"], dtype=np.float32)
    Wc = np.asarray(inputs["Wc"], dtype=np.float32)
    bc = np.asarray(inputs["bc"], dtype=np.float32)

    with_bias = bool(np.any(bvec))
    bf = ml_dtypes.bfloat16
    hT_np = np.ascontiguousarray(
        hidden.reshape(R, H).T.reshape(KT, 128, R)).astype(bf)
    wcT_np = np.ascontiguousarray(Wc.reshape(1, H).T.reshape(KT, 128, 1))
    bc_np = bc.reshape(1, 1)

    in_maps = []
    for c in range(N_CORES):
        vs = slice(c * VL, (c + 1) * VL)
        bs = slice(c * BL, (c + 1) * BL)
        wT_np = np.ascontiguousarray(W[vs].T.reshape(KT, 128, VL)).astype(bf)
        dT_np = np.ascontiguousarray(
            dec[:, bs, :].reshape(RL, H).T.reshape(KT, 128, RL))
        # attnT[s, b*T + t] = attn[t, c*BL+b, s]; hi/lo bf16 split
        at = np.ascontiguousarray(
            attn[:, bs, :].transpose(2, 1, 0).reshape(S, BL * T))
        at_hi = at.astype(bf)
        at_lo = (at - at_hi.astype(np.float32)).astype(bf)
        attnT_np = np.ascontiguousarray(np.stack([at_hi, at_lo]))
        idx_np = np.ascontiguousarray(c2e[:, bs]).astype(np.float32)
        m = {"dT": dT_np, "wcT": wcT_np, "bc": bc_np,
             "attnT": attnT_np, "idx": idx_np}
        for k in range(KT):
            m[f"hT{k}"] = np.ascontiguousarray(hT_np[k])
            m[f"wT{k}"] = np.ascontiguousarray(wT_np[k])
        if with_bias:
            m["brow"] = bvec[vs].reshape(1, VL).astype(bf)
        in_maps.append(m)

    nc = _get_nc(with_bias)
    res = run_bass_kernel_spmd(nc, in_maps, core_ids=list(range(N_CORES)))

    out = np.empty((T, B, VT + VE), dtype=np.float32)
    for c in range(N_CORES):
        r = res.results[c]
        out[:, :, c * VL:(c + 1) * VL] = r["out_tgt"].reshape(T, B, VL)
        out[:, c * BL:(c + 1) * BL, VT:] = r["out_ext"].transpose(1, 0, 2)
    return out



# revision 17
# speedup vs baseline: 1.5132x; 1.5132x over previous
"""Trainium2 Bass kernel for nn_CopyGenerator (scatter_memory).

Computation (see the reference):
  out_tgt = log_softmax(hidden @ W.T + b)                    [T,B,VT]
  gate1m  = 1 - sigmoid(dec @ Wc.T + bc)                     [T,B]
  ext[t,b,v] = gate1m[t,b] * sum_s attn[t,b,s]*(idx[s,b]==v), idx==UNK masked
  out_ext = log(clip(ext, 1e-3, 1-1e-3))                     [T,B,VE]
  out = concat([out_tgt, out_ext], -1)

Sharding (8 cores): column-parallel over tgt vocab for the big matmul
(4000 cols/core, all 3200 rows); data-parallel over batch for the ext
scatter part (8 batch elements/core).

Key implementation choices:
  - Main matmul in fp8 e4m3 with DoubleRow perf mode (2 k-planes per
    instruction, 0.5 cycles/row). W pre-scaled x64, hidden x8 to stay in
    the fp8 normal range; the 1/512 descale folds into the exp scale and
    the psum->x copy.
  - Outputs written to HBM as bf16 and upconverted in numpy (rel rounding
    2^-9, far inside the 2e-2 gate); halves write traffic.
  - log_softmax denominator: per-chunk partial row sums AllGathered across
    cores (gather + local sum is 15us vs 28us for AllReduce in the cost
    model), with a small final chunk to shorten the tail.
  - out_tgt span A (first W_A cols) is recovered as Ln(E * 1/total) from
    the exp output E on ACT; span B is psum->bf16 x copies + a 4x-mode DVE
    add of -lse. This balances ACT vs DVE.
  - Ext scatter: aw.T @ onehot(idx) on PE (bf16 hi/lo attn split for
    accuracy), batch-PAIRED at partition offsets 0/64 so Ln/clip process
    two batch elements per instruction. onehot is precomputed in numpy and
    DMA'd in (DMA has slack; saves Pool/DVE work).
"""

import sys

if "/opt/trn_rl_repo" not in sys.path:
    sys.path.insert(0, "/opt/trn_rl_repo")

from contextlib import ExitStack

import ml_dtypes
import numpy as np

import concourse.bass as bass
import concourse.mybir as mybir
import concourse.tile as tile
from concourse import bacc
from concourse.bass_utils import run_bass_kernel_spmd

F32 = mybir.dt.float32
BF16 = mybir.dt.bfloat16
FP8 = mybir.dt.float8e4
AF = mybir.ActivationFunctionType
OP = mybir.AluOpType
DR = mybir.MatmulPerfMode.DoubleRow

T, B, S, H = 50, 64, 100, 512
VT, VE = 32000, 5000
N_CORES = 8
VL = VT // N_CORES       # 4000 vocab cols per core
BL = B // N_CORES        # 8 batch per core (ext part)
R = T * B                # 3200 rows
RL = T * BL              # 400 rows (ext part)
KT = H // 128            # 4 k-tiles (gate, fp32)
KP = 2                   # fp8 DoubleRow k-tile pairs (256 each)
MT = R // 128            # 25 m-tiles
NB = 8                   # 500-wide n-blocks per m-tile
NW = 500
HB = 4                   # n-blocks per psum half
CHUNKS = [6, 6, 6, 5, 2]  # m-tiles per lse chunk (small tail)
W_A = 500                # Ln(E*s) span (rest: x-copy + DVE add)
WB_A = W_A // NW         # blocks in span A
SCALE = 1.0 / 512.0      # fp8 descale (h x8, W x64)
NPAIR = BL // 2          # 4 ext batch pairs
EXT_SPANS = [(0, 4), (4, 8), (8, 10)]   # ext n-block spans (x500)

LOG_LO = float(np.log(0.001))
LOG_HI = float(np.log(1.0 - 0.001))

_CACHE = {}


def _dedupe_act_table_loads(nc):
    """Collapse activation-table thrash: point every load at a table that
    serves its following activations when one exists, then drop loads that
    re-load the already-loaded table. Saves ~1.3us per removed load on ACT."""
    from concourse.hw_specs import get_activation_tables
    tables = list(get_activation_tables(nc.m.arch).items())
    for blk in nc.m.functions[0].blocks:
        insts = blk.instructions
        loads = [(i, inst) for i, inst in enumerate(insts)
                 if isinstance(inst, mybir.InstLoadActFuncSet)]
        if not loads:
            continue
        for li, (pos, inst) in enumerate(loads):
            end = loads[li + 1][0] if li + 1 < len(loads) else len(insts)
            funcs = {s.func for s in insts[pos:end]
                     if isinstance(s, mybir.InstActivation)}
            if not funcs:
                continue
            want = funcs | {AF.Exp, AF.Ln, AF.Identity, AF.Copy}
            pick = None
            for tid, (name, fs) in enumerate(tables):
                if want <= fs:
                    pick = tid
                    break
            if pick is None:
                for tid, (name, fs) in enumerate(tables):
                    if funcs <= fs:
                        pick = tid
                        break
            if pick is not None:
                inst.act_func_set_id = pick
        cur = None
        to_drop = []
        for pos, inst in loads:
            if cur is not None and inst.act_func_set_id == cur:
                si = inst.sync_info
                clean = si is None or (not si.on_wait and not si.on_update)
                if clean:
                    to_drop.append(inst)
                    continue
            cur = inst.act_func_set_id
        for inst in to_drop:
            insts.remove(inst)


def _build(with_bias):
    nc = bacc.Bacc("TRN2", target_bir_lowering=False, debug=False,
                   num_devices=N_CORES)

    # fp8 weights/hidden, DoubleRow layout: [k(128), i(2), free]
    wT2 = [nc.dram_tensor(f"wT2_{t}", [128, 2 * VL], FP8, kind="ExternalInput").ap()
           for t in range(KP)]
    hT2 = [nc.dram_tensor(f"hT2_{t}", [128, 2 * R], FP8, kind="ExternalInput").ap()
           for t in range(KP)]
    if with_bias:
        brow = nc.dram_tensor("brow", [1, VL], BF16, kind="ExternalInput").ap()
    dT = nc.dram_tensor("dT", [KT, 128, RL], F32, kind="ExternalInput").ap()
    wcT = nc.dram_tensor("wcT", [KT, 128, 1], F32, kind="ExternalInput").ap()
    bias_col = nc.dram_tensor("bias_col", [128, 1], F32, kind="ExternalInput").ap()
    attnT = nc.dram_tensor("attnT", [2, S, BL * T], BF16, kind="ExternalInput").ap()
    ohd = nc.dram_tensor("oh", [BL, S, VE], BF16, kind="ExternalInput").ap()

    out_tgt = nc.dram_tensor("out_tgt", [R, VL], BF16, kind="ExternalOutput").ap()
    out_ext = nc.dram_tensor("out_ext", [BL * T, VE], BF16, kind="ExternalOutput").ap()

    NCH = len(CHUNKS)
    cc_in = [nc.dram_tensor(f"cc_in{g}", [128, 2 * CHUNKS[g]], F32).ap()
             for g in range(NCH)]
    cc_out = [nc.dram_tensor(f"cc_out{g}", [N_CORES, 128, 2 * CHUNKS[g]], F32,
                             addr_space="Shared").ap()
              for g in range(NCH)]

    core_ids = list(range(N_CORES))

    with tile.TileContext(nc) as tc, ExitStack() as ctx:
        const = ctx.enter_context(tc.tile_pool(name="const", bufs=1))
        combp = ctx.enter_context(tc.tile_pool(name="comb", bufs=12))
        escr = ctx.enter_context(tc.tile_pool(name="escr", bufs=2))
        extp = ctx.enter_context(tc.tile_pool(name="exts", bufs=3))
        ohp = ctx.enter_context(tc.tile_pool(name="oh", bufs=4))
        statp = ctx.enter_context(tc.tile_pool(name="stat", bufs=2))
        psp = ctx.enter_context(tc.tile_pool(name="ps", bufs=2, space="PSUM"))

        # ---- persistent SBUF loads (order = SP issue order; weights first
        # so PE can start immediately) ----
        wT2_sb = [const.tile([128, 2 * VL], FP8, name=f"w2s{t}") for t in range(KP)]
        hT2_sb = [const.tile([128, 2 * R], FP8, name=f"h2s{t}") for t in range(KP)]
        for t in range(KP):
            nc.sync.dma_start(wT2_sb[t][:], wT2[t])
            nc.sync.dma_start(hT2_sb[t][:], hT2[t])
        dT_sb = const.tile([128, KT * RL], F32)
        for k in range(KT):
            nc.sync.dma_start(dT_sb[:, k * RL:(k + 1) * RL], dT[k])
        wcT_sb = const.tile([128, KT], F32)
        for k in range(KT):
            nc.sync.dma_start(wcT_sb[:, k:k + 1], wcT[k])
        bias_sb = const.tile([128, 1], F32)
        nc.sync.dma_start(bias_sb[:], bias_col[:])
        if with_bias:
            b_sb = const.tile([1, VL], BF16)
            nc.sync.dma_start(b_sb[:], brow[:])
            ones_sb = const.tile([1, 128], BF16)
            nc.vector.memset(ones_sb[:], 1.0)
        attnT_sb = const.tile([S, 2 * BL * T], BF16)
        nc.sync.dma_start(attnT_sb[:, :BL * T], attnT[0])
        nc.sync.dma_start(attnT_sb[:, BL * T:], attnT[1])

        # 3D DoubleRow views: [128, i(2), free]
        wv = [wT2_sb[t][:].rearrange("p (i v) -> p i v", i=2) for t in range(KP)]
        hv = [hT2_sb[t][:].rearrange("p (i r) -> p i r", i=2) for t in range(KP)]

        # ---- copy gate: g1m[64j+t, p] = 1 - sigmoid(dec[t,2p+j] . Wc + bc) ----
        g1m = const.tile([128, NPAIR], F32)
        sig = const.tile([128, NPAIR], F32)

        def emit_gate():
            nc.vector.memset(g1m[:], 1.0)
            gp = psp.tile([128, 2048], F32, tag="ps")
            for p in range(NPAIR):
                for j in range(2):
                    b = 2 * p + j
                    o = gp[64 * j:64 * j + T, p * 512:p * 512 + 1]
                    for k in range(KT):
                        lhs = dT_sb[:, k * RL + b: k * RL + b + (T - 1) * BL + 1: BL]
                        nc.tensor.matmul(o, lhsT=lhs, rhs=wcT_sb[:, k:k + 1],
                                         start=(k == 0), stop=(k == KT - 1))
            gpv = gp[:].rearrange("p (b n) -> p b n", b=NPAIR)[0:114, :, 0:1]
            nc.scalar.activation(sig[0:114, :].rearrange("p (b n) -> p b n", n=1),
                                 gpv, AF.Sigmoid, bias=bias_sb[0:114, 0:1])
            nc.vector.tensor_scalar(g1m[0:114, :], sig[0:114, :], -1.0, 1.0,
                                    OP.mult, OP.add)

        # ---- ext pair emitter ----
        def emit_oh_load(p):
            tls = []
            for j in range(2):
                t_oh = ohp.tile([S, VE], BF16, tag="oh")
                nc.sync.dma_start(t_oh[:], ohd[2 * p + j])
                tls.append(t_oh)
            return tls

        def emit_ext_span(p, span, oh_tiles):
            b0, b1 = EXT_SPANS[span]
            w = (b1 - b0) * NW
            eps = psp.tile([128, 2048], F32, tag="ps")
            for j in range(2):
                b = 2 * p + j
                for hl in range(2):
                    for nb in range(b1 - b0):
                        nc.tensor.matmul(
                            eps[64 * j:64 * j + T, nb * 512:nb * 512 + NW],
                            lhsT=attnT_sb[:, (hl * BL + b) * T:(hl * BL + b + 1) * T],
                            rhs=oh_tiles[j][:, (b0 + nb) * NW:(b0 + nb + 1) * NW],
                            start=(hl == 0), stop=(hl == 1))
            st = extp.tile([128, 2048], BF16, tag="exts")
            nbk = b1 - b0
            src = eps[:].rearrange("p (b n) -> p b n", b=4)[0:114, :nbk, :NW]
            dst = st[:, :w].rearrange("p (b n) -> p b n", n=NW)[0:114]
            nc.scalar.activation(dst, src, AF.Ln, scale=g1m[0:114, p:p + 1])
            nc.vector.tensor_scalar(st[0:114, :w], st[0:114, :w],
                                    LOG_LO, LOG_HI, OP.max, OP.min)
            for j in range(2):
                nc.sync.dma_start(
                    out_ext[(2 * p + j) * T:(2 * p + j + 1) * T,
                            b0 * NW:b1 * NW],
                    st[64 * j:64 * j + T, :w])

        # ---- main: logits -> E/x, chunked allgather lse, outputs ----
        sums, gaths, combs = {}, {}, {}

        def emit_produce(g, j, m):
            ps_halves = []
            comb = combp.tile([128, NB * NW], BF16, tag="comb")
            combs[(g, j)] = comb
            for half in range(2):
                ps = psp.tile([128, 2048], F32, tag="ps")
                ps_halves.append(ps)
                for nb in range(HB):
                    n = half * HB + nb
                    for t in range(KP):
                        last = (t == KP - 1) and not with_bias
                        nc.tensor.matmul(
                            ps[:, nb * 512:nb * 512 + NW],
                            lhsT=hv[t][:, :, m * 128:(m + 1) * 128],
                            rhs=wv[t][:, :, n * NW:(n + 1) * NW],
                            start=(t == 0), stop=last, perf_mode=DR)
                    if with_bias:
                        nc.tensor.matmul(
                            ps[:, nb * 512:nb * 512 + NW],
                            lhsT=ones_sb[:],
                            rhs=b_sb[:, n * NW:(n + 1) * NW],
                            start=False, stop=True)
            # ACT: exp halves -> scratch (E span A copied out by DVE below,
            # so the x-copies never serialize behind the exps), accum sums
            sg = sums[g]
            psA = ps_halves[0][:].rearrange("p (b n) -> p b n", b=HB)[:, :, :NW]
            e_sA = escr.tile([128, HB * NW], BF16, tag="escr", name=f"esA{j}")
            nc.scalar.activation(e_sA[:].rearrange("p (b n) -> p b n", b=HB),
                                 psA, AF.Exp, scale=SCALE,
                                 accum_out=sg[:, 2 * j:2 * j + 1])
            psB = ps_halves[1][:].rearrange("p (b n) -> p b n", b=HB)[:, :, :NW]
            e_sB = escr.tile([128, HB * NW], BF16, tag="escr", name=f"esB{j}")
            nc.scalar.activation(e_sB[:].rearrange("p (b n) -> p b n", b=HB),
                                 psB, AF.Exp, scale=SCALE,
                                 accum_out=sg[:, 2 * j + 1:2 * j + 2])
            # DVE: x = psum * (1/512) -> bf16 (span B; independent of exps)
            if WB_A < HB:
                nc.vector.tensor_scalar(
                    comb[:, W_A:HB * NW].rearrange("p (b n) -> p b n",
                                                   b=HB - WB_A),
                    psA[:, WB_A:, :], SCALE, None, OP.mult)
            nc.vector.tensor_scalar(
                comb[:, HB * NW:].rearrange("p (b n) -> p b n", b=HB),
                psB, SCALE, None, OP.mult)
            # DVE 4x: keep E span A for the post-gather Ln
            if W_A > 0:
                nc.vector.tensor_copy(comb[:, :W_A], e_sA[:, :W_A])

        def emit_gather(g):
            W = 2 * CHUNKS[g]
            nc.sync.dma_start(cc_in[g][:], sums[g][:, :W])
            nc.gpsimd.collective_compute(
                "AllGather", OP.bypass, replica_groups=[core_ids],
                ins=[cc_in[g][:]], outs=[cc_out[g][:]])
            gath = statp.tile([128, N_CORES * 2 * max(CHUNKS)], F32, tag="gath")
            for c in range(N_CORES):
                nc.sync.dma_start(gath[:, c * W:(c + 1) * W], cc_out[g][c])
            t4 = statp.tile([128, 4 * 2 * max(CHUNKS)], F32, tag="t4")
            nc.vector.tensor_tensor(t4[:, :4 * W], gath[:, :4 * W],
                                    gath[:, 4 * W:8 * W], OP.add)
            t2 = statp.tile([128, 2 * 2 * max(CHUNKS)], F32, tag="t2")
            nc.vector.tensor_tensor(t2[:, :2 * W], t4[:, :2 * W],
                                    t4[:, 2 * W:4 * W], OP.add)
            ts1 = statp.tile([128, 2 * max(CHUNKS)], F32, tag="ts1")
            nc.vector.tensor_tensor(ts1[:, :W], t2[:, :W], t2[:, W:2 * W],
                                    OP.add)
            # fold per-half sums: tot[:, j] = ts1[2j] + ts1[2j+1]
            tot = statp.tile([128, max(CHUNKS)], F32, tag="tot")
            th = ts1[:, :W].rearrange("p (j two) -> p j two", two=2)
            nc.vector.tensor_tensor(
                tot[:, :CHUNKS[g]].rearrange("p (j one) -> p j one", one=1),
                th[:, :, 0:1], th[:, :, 1:2], OP.add)
            lnt = statp.tile([128, 2 * max(CHUNKS)], F32, tag="lnt")
            nc.scalar.activation(lnt[:, :CHUNKS[g]], tot[:, :CHUNKS[g]], AF.Ln)
            neglse = statp.tile([128, max(CHUNKS)], F32, tag="lse")
            nc.vector.tensor_scalar(neglse[:, :CHUNKS[g]], lnt[:, :CHUNKS[g]],
                                    -1.0, None, OP.mult)
            s = statp.tile([128, max(CHUNKS)], F32, tag="s")
            nc.vector.reciprocal(s[:, :CHUNKS[g]], tot[:, :CHUNKS[g]])
            gaths[g] = (neglse, s)

        def emit_outgen(g, j, m):
            neglse, s = gaths[g]
            comb = combs.pop((g, j))
            if W_A > 0:
                nc.scalar.activation(comb[:, :W_A], comb[:, :W_A], AF.Ln,
                                     scale=s[:, j:j + 1])
            nc.vector.tensor_scalar(comb[:, W_A:], comb[:, W_A:],
                                    neglse[:, j:j + 1], None, OP.add)
            nc.sync.dma_start(out_tgt[m * 128:(m + 1) * 128, :], comb[:])

        # ---- schedule: ext spans + outgen interleave one-per-produce-tile ----
        OFF = 3  # delay outgen(g-1) so it never stalls on the gather latency
        starts = np.cumsum([0] + CHUNKS).tolist()
        spanq = [(p, s) for p in range(NPAIR) for s in range(len(EXT_SPANS))]
        oh_cur = emit_oh_load(0)
        oh_nxt = emit_oh_load(1)
        tix = 0
        for g in range(NCH):
            sums[g] = statp.tile([128, 2 * max(CHUNKS)], F32, tag="sums",
                                 name=f"sums{g}")
            prev = [(g - 1, jj, starts[g - 1] + jj)
                    for jj in range(CHUNKS[g - 1])] if g > 0 else []
            for j in range(CHUNKS[g]):
                emit_produce(g, j, starts[g] + j)
                if tix == 0:
                    emit_gate()
                elif spanq:
                    p, s = spanq.pop(0)
                    emit_ext_span(p, s, oh_cur)
                    if s == len(EXT_SPANS) - 1:
                        oh_cur = oh_nxt
                        oh_nxt = (emit_oh_load(p + 2)
                                  if p + 2 < NPAIR else None)
                tix += 1
                if 0 <= j - OFF < len(prev):
                    emit_outgen(*prev[j - OFF])
            # gather BEFORE the leftover outgens: its tiny cc_in DMA must not
            # queue behind their 2.8us-each out_tgt writes
            emit_gather(g)
            for jj in range(max(0, CHUNKS[g] - OFF), len(prev)):
                emit_outgen(*prev[jj])
        g = NCH - 1
        for jj in range(CHUNKS[g]):
            emit_outgen(g, jj, starts[g] + jj)

    nc.compile()
    _dedupe_act_table_loads(nc)
    return nc


def _get_nc(with_bias=False):
    key = ("nc", with_bias)
    if key not in _CACHE:
        _CACHE[key] = _build(with_bias)
    return _CACHE[key]


def kernel(**inputs):
    hidden = np.asarray(inputs["hidden"], dtype=np.float32)
    dec = np.asarray(inputs["dec_rnn_output"], dtype=np.float32)
    attn = np.asarray(inputs["attn"], dtype=np.float32)
    c2e = np.asarray(inputs["copy_to_ext"])
    W = np.asarray(inputs["W"], dtype=np.float32)
    bvec = np.asarray(inputs["b"], dtype=np.float32)
    Wc = np.asarray(inputs["Wc"], dtype=np.float32)
    bc = np.asarray(inputs["bc"], dtype=np.float32)

    with_bias = bool(np.any(bvec))
    bf = ml_dtypes.bfloat16
    e4 = ml_dtypes.float8_e4m3

    # fp8 DoubleRow layouts: [t][k(128), i(2)*free], h index = t*256+i*128+k
    h8 = (hidden.reshape(R, H) * 8.0).T.reshape(KP, 2, 128, R).astype(e4)
    hT2_np = [np.ascontiguousarray(h8[t].transpose(1, 0, 2).reshape(128, 2 * R))
              for t in range(KP)]
    wcT_np = np.ascontiguousarray(Wc.reshape(1, H).T.reshape(KT, 128, 1))
    bias_np = np.full((128, 1), float(bc.reshape(-1)[0]), np.float32)

    # onehot (UNK masked): oh[b, s, v] = (c2e[s, b] == v) & (v != 0)
    vids = np.arange(VE, dtype=np.int32)
    vids[0] = -1

    in_maps = []
    for c in range(N_CORES):
        vs = slice(c * VL, (c + 1) * VL)
        bs = slice(c * BL, (c + 1) * BL)
        w64 = (W[vs] * 64.0).T.reshape(KP, 2, 128, VL).astype(e4)
        wT2_np = [np.ascontiguousarray(
            w64[t].transpose(1, 0, 2).reshape(128, 2 * VL)) for t in range(KP)]
        dT_np = np.ascontiguousarray(
            dec[:, bs, :].reshape(RL, H).T.reshape(KT, 128, RL))
        at = np.ascontiguousarray(
            attn[:, bs, :].transpose(2, 1, 0).reshape(S, BL * T))
        at_hi = at.astype(bf)
        at_lo = (at - at_hi.astype(np.float32)).astype(bf)
        attnT_np = np.ascontiguousarray(np.stack([at_hi, at_lo]))
        oh_np = np.ascontiguousarray(
            (c2e[:, bs].T[:, :, None] == vids[None, None, :]).astype(bf))
        m = {"dT": dT_np, "wcT": wcT_np, "bias_col": bias_np,
             "attnT": attnT_np, "oh": oh_np}
        for t in range(KP):
            m[f"hT2_{t}"] = hT2_np[t]
            m[f"wT2_{t}"] = wT2_np[t]
        if with_bias:
            m["brow"] = bvec[vs].reshape(1, VL).astype(bf)
        in_maps.append(m)

    nc = _get_nc(with_bias)
    res = run_bass_kernel_spmd(nc, in_maps, core_ids=list(range(N_CORES)))

    out = np.empty((T, B, VT + VE), dtype=np.float32)
    for c in range(N_CORES):
        r = res.results[c]
        out[:, :, c * VL:(c + 1) * VL] = (
            r["out_tgt"].astype(np.float32).reshape(T, B, VL))
        out[:, c * BL:(c + 1) * BL, VT:] = (
            r["out_ext"].astype(np.float32).reshape(BL, T, VE).transpose(1, 0, 2))
    return out
